# revision 30
# baseline (speedup 1.0000x reference)
"""Causal multi-head attention (RoPE) TRN2 Bass kernel.

Problem: x[2,2048,2048] fp32, Wq/Wk/Wv/Wo [2048,2048], 16 heads, d_k=128,
causal softmax attention with interleaved RoPE, out = attn_out @ Wo.

Sharding (8 cores): core = b*4 + g handles batch b and head group g
(4 heads = 512 feature columns). Wq/Wk/Wv split column-wise, Wo row-wise;
the "all-reduce" after the output projection is done on the host by summing
the 4 partial outputs per batch (gather/unshard step).

All matmul operands are bf16 (inputs cast on the host, intermediates cast
at the PSUM drain): with bf16 weights the PE's LDWEIGHTS (~112ns) hides
under the 512-row matmul stream (~213ns), where the fp32r version's ~224ns
weight load serialized ~60ns per matmul. PSUM accumulation stays fp32.
Weights (Wq/Wk/Wv/Wo) and the RoPE tables are SBUF-resident, loaded once;
xT streams per 512-row chunk through a double-buffered tile (prefetched a
full chunk ahead).

Device kernel (per core), per 512-row chunk j:
  section 1: QT/KT = (x @ Wq/Wk)^T via lhsT=W tiles, rhs=xT, RoPE fused on
    VectorE; V = x @ Wv interleaved two k-steps per projection group; the
    PREVIOUS chunk's output projection (O @ Wo) interleaved as well.
  section 2: causal attention for q-tile j, scores computed transposed
    (S^T[k,q]) so softmax weights feed attn@V without transposes; exp on
    ScalarE (no max subtraction: scores are O(5)); rs/o accumulation
    matmuls trail the S matmuls by LAG slots in one flat software-pipelined
    stream across all four heads. Diagonal blocks are TRIMMED: for diagonal
    block d the first 128*d query columns are fully masked, so S/rs/o
    matmuls run on the live [128d:512] range only (bf16 keeps 1 cycle/row
    even at narrow widths) and no zero-fill is needed. Row sums: diagonal
    blocks use an all-ones lhsT matmul directly; history blocks are first
    summed in groups of 4 on VectorE (independent bf16 adds) with one
    ones-matmul per group, quartering the PE row-sum cost. 1/rs uses the
    single-op DVE reciprocal_approx_fast (~5x faster than the exact
    RECIPROCAL).

DMA: one Sync-queue stream for inputs in first-use order (weights land ~a
group ahead of their matmuls); mid-kernel output tiles ride the idle
GpSimd software-DGE queue; the final chunk's outputs use the Sync HWDGE
queue (idle by then, and its completion skips the slow SWDGE drain).

RoPE pair trick: scores are invariant under any permutation of d_k applied
to both Q and K, so W columns are permuted per head to [even..., odd...] on
the host; the rotate pairs then live 64 partitions apart (the sin-term
muls read ps at a 64-partition offset directly), and cosT/sinT are
permuted/sign-baked to match.
"""

import math
import sys

sys.path.insert(0, "/opt/trn_rl_repo")

import numpy as np

D_MODEL = 2048
SEQ = 2048
BATCH = 2
N_CORES = 8
HEADS_PER_CORE = 4
GCOLS = HEADS_PER_CORE * 128  # 512 feature columns per core
KB = D_MODEL // 128  # 16 contraction blocks
N_CHUNKS = SEQ // 512  # 4
SCALE = 1.0 / math.sqrt(128.0)

_CACHE = {}


def _build_program():
    import concourse.mybir as mybir
    import concourse.tile as tile
    from concourse import bacc

    F = mybir.dt.float32
    BF = mybir.dt.bfloat16
    AF = mybir.ActivationFunctionType

    nc = bacc.Bacc("TRN2", target_bir_lowering=False, debug=False,
                   num_devices=N_CORES)

    xT_d = nc.dram_tensor("xT", (D_MODEL, SEQ), BF, kind="ExternalInput").ap()
    Wq_d = nc.dram_tensor("Wq", (D_MODEL, GCOLS), BF, kind="ExternalInput").ap()
    Wk_d = nc.dram_tensor("Wk", (D_MODEL, GCOLS), BF, kind="ExternalInput").ap()
    Wv_d = nc.dram_tensor("Wv", (D_MODEL, GCOLS), BF, kind="ExternalInput").ap()
    Wo_d = nc.dram_tensor("Wo", (GCOLS, D_MODEL), BF, kind="ExternalInput").ap()
    cosT_d = nc.dram_tensor("cosT", (128, SEQ), F, kind="ExternalInput").ap()
    sinT_d = nc.dram_tensor("sinT", (128, SEQ), F, kind="ExternalInput").ap()
    mask_d = nc.dram_tensor("mask", (128, 128), BF, kind="ExternalInput").ap()
    # partial outputs are summed across 4 cores on the host (in f64), so
    # bf16 partials cost ~0.2% relative error but halve the output traffic
    # that dominates the end-of-kernel DMA drain.
    out_d = nc.dram_tensor("out", (SEQ, D_MODEL), BF, kind="ExternalOutput").ap()

    with tile.TileContext(nc) as tc:
        with tc.tile_pool(name="resid", bufs=1) as resid, \
             tc.tile_pool(name="xtp", bufs=2) as xtp, \
             tc.tile_pool(name="qtp", bufs=2) as qtp, \
             tc.tile_pool(name="otp", bufs=1) as otp, \
             tc.tile_pool(name="ep", bufs=12) as ep, \
             tc.tile_pool(name="gp", bufs=5) as gp, \
             tc.tile_pool(name="ropep", bufs=3) as ropep, \
             tc.tile_pool(name="rcp", bufs=2) as rcp, \
             tc.tile_pool(name="outp", bufs=3) as outp, \
             tc.tile_pool(name="psA", bufs=4, space="PSUM") as psA, \
             tc.tile_pool(name="psB", bufs=4, space="PSUM") as psB:

            ones = resid.tile([128, 128], BF, tag="ones")
            nc.vector.memset(ones[:], 1.0)
            mask_sb = resid.tile([128, 128], BF, tag="mask")
            KT = resid.tile([128, HEADS_PER_CORE, SEQ], BF, tag="KT")
            V = resid.tile([128, KB, GCOLS], BF, tag="V")
            wo = resid.tile([128, HEADS_PER_CORE, D_MODEL], BF, tag="wo")
            wq = resid.tile([128, KB, GCOLS], BF, tag="wq")
            wk = resid.tile([128, KB, GCOLS], BF, tag="wk")
            wv = resid.tile([128, KB, GCOLS], BF, tag="wv")
            cosT = resid.tile([128, SEQ], F, tag="cosT")
            sinT = resid.tile([128, SEQ], F, tag="sinT")

            xT_r = xT_d.rearrange("(ko p) s -> p ko s", p=128)
            Wq_r = Wq_d.rearrange("(ko p) m -> p ko m", p=128)
            Wk_r = Wk_d.rearrange("(ko p) m -> p ko m", p=128)
            Wv_r = Wv_d.rearrange("(ko p) m -> p ko m", p=128)

            def emit_wo_step(jprev, prev_ot, m, n, alt=False, last=False):
                ps = psA.tile([128, 512], F, tag="flow", name="wops")
                for c in range(HEADS_PER_CORE):
                    nc.tensor.matmul(
                        ps[:], prev_ot[:, c, m * 128:(m + 1) * 128],
                        wo[:, c, n * 512:(n + 1) * 512],
                        start=(c == 0), stop=(c == 3))
                ob = outp.tile([128, 512], BF, tag="ob")
                if alt:
                    nc.scalar.copy(ob[:], ps[:])
                else:
                    nc.vector.tensor_copy(ob[:], ps[:])
                # mid-kernel output DMAs ride the (otherwise idle) GpSimd
                # software-DGE queue so they never serialize behind the input
                # stream; the final chunk's go on the Sync HWDGE queue (idle
                # by then) whose completion doesn't need the slow SWDGE drain
                dst = out_d[(4 * jprev + m) * 128:(4 * jprev + m + 1) * 128,
                            n * 512:(n + 1) * 512]
                if last:
                    nc.sync.dma_start(dst, ob[:])
                else:
                    nc.gpsimd.dma_start(dst, ob[:])

            prev_ot = None

            for j in range(N_CHUNKS):
                ssl = slice(j * 512, (j + 1) * 512)

                if j == 0:
                    # startup: one queue, DMAs issued in first-use order so
                    # each QK group's weights land ~a group ahead of its
                    # matmuls (multiple queues lose this priority ordering)
                    xt = xtp.tile([128, KB, 512], BF, tag="xt")
                    nc.sync.dma_start(wq[:, 0:1, 0:128], Wq_r[:, 0:1, 0:128])
                    nc.sync.dma_start(xt[:, 0:1], xT_r[:, 0:1, ssl])
                    nc.sync.dma_start(wq[:, 1:KB, 0:128], Wq_r[:, 1:KB, 0:128])
                    nc.sync.dma_start(xt[:, 1:4], xT_r[:, 1:4, ssl])
                    nc.sync.dma_start(xt[:, 4:8], xT_r[:, 4:8, ssl])
                    nc.sync.dma_start(wv[:, 0:2], Wv_r[:, 0:2])
                    nc.sync.dma_start(cosT[:, 0:512], cosT_d[:, 0:512])
                    nc.sync.dma_start(sinT[:, 0:512], sinT_d[:, 0:512])
                    nc.sync.dma_start(xt[:, 8:KB], xT_r[:, 8:KB, ssl])
                    for g in range(1, 8):
                        wdst, wsrc = (wq, Wq_r) if g < 4 else (wk, Wk_r)
                        c0 = (g % 4) * 128
                        nc.sync.dma_start(wdst[:, :, c0:c0 + 128],
                                          wsrc[:, :, c0:c0 + 128])
                        nc.sync.dma_start(wv[:, 2 * g:2 * g + 2],
                                          Wv_r[:, 2 * g:2 * g + 2])
                    nc.sync.dma_start(mask_sb[:], mask_d)
                    nc.sync.dma_start(cosT[:, 512:SEQ], cosT_d[:, 512:SEQ])
                    nc.sync.dma_start(sinT[:, 512:SEQ], sinT_d[:, 512:SEQ])
                else:
                    xt = xt_next

                if j < N_CHUNKS - 1:
                    # prefetch next chunk's xT a full chunk ahead
                    xt_next = xtp.tile([128, KB, 512], BF, tag="xt")
                    nc.sync.dma_start(
                        xt_next[:], xT_r[:, :, (j + 1) * 512:(j + 2) * 512])

                cos_t = cosT[:, ssl]
                sin_t = sinT[:, ssl]
                qt = qtp.tile([128, HEADS_PER_CORE, 512], BF, tag="qt")

                # --- Q/K projections + RoPE (outputs transposed: [d_k, s]),
                # with the V projection's k-steps interleaved between groups
                # and the previous chunk's output projection as well. ---
                vps = [psB.tile([128, 512], F, tag="hold", name=f"vps{m}")
                       for m in range(4)]
                groups = [(qt, True, wq, m) for m in range(HEADS_PER_CORE)]
                groups += [(KT, False, wk, m) for m in range(HEADS_PER_CORE)]
                # group 0 carries no V/Wo work: the V matmuls (psB) and the
                # first Wo steps (prev_ot) would otherwise stall the in-order
                # PE queue on the previous chunk's last norms draining on
                # VectorE at the chunk boundary
                SCHED = [(), (0, 1), (2, 3), (4, 5), (6, 7), (8, 9),
                         (10, 11, 12), (13, 14, 15)]
                for g, (dst, is_q, w, m) in enumerate(groups):
                    msl = slice(m * 128, (m + 1) * 128)
                    ps = psA.tile([128, 512], F, tag="flow")
                    for k in range(KB):
                        nc.tensor.matmul(ps[:], w[:, k, msl], xt[:, k],
                                         start=(k == 0), stop=(k == KB - 1))
                    # V k-steps interleaved between groups
                    for k in SCHED[g]:
                        for m2 in range(4):
                            nc.tensor.matmul(
                                vps[m2][:],
                                xt[:, k, m2 * 128:(m2 + 1) * 128], wv[:, k],
                                start=(k == 0), stop=(k == KB - 1))
                    # Wo output-projection steps for the previous chunk
                    if prev_ot is not None:
                        for t in SCHED[g]:
                            emit_wo_step(j - 1, prev_ot, t // 4, t % 4)
                    # rotate-halves trick: the sin term reads ps with a
                    # 64-partition offset directly (no staging copies)
                    rot = ropep.tile([128, 512], F, tag="rot")
                    out_ap = dst[:, m, :] if is_q else dst[:, m, ssl]
                    nc.vector.tensor_mul(out_ap, ps[:], cos_t)
                    nc.vector.tensor_mul(rot[:64, :], ps[64:128, :],
                                         sin_t[:64, :])
                    nc.vector.tensor_mul(rot[64:128, :], ps[:64, :],
                                         sin_t[64:128, :])
                    nc.vector.tensor_add(out_ap, out_ap, rot[:])
                for m in range(4):
                    nc.any.tensor_copy(V[:, 4 * j + m, :], vps[m][:])

                if j == 0:
                    # Wo is first needed ~70us in; keep it off the startup
                    # critical path but loaded well before the Wo section.
                    nc.sync.dma_start(
                        wo[:], Wo_d.rearrange("(c p) n -> p c n", p=128))

                # --- causal attention for q-tile j ---
                # kb order per head: diagonal blocks first (their exp+mask
                # chain is the longest), then the fully-unmasked history
                # blocks. One flat software-pipelined stream across all four
                # heads: the rs/o accumulation matmuls for E(i) trail the S
                # matmuls by LAG slots. Diagonal block d only computes the
                # live query range [128d:512].
                ot = otp.tile([128, HEADS_PER_CORE, 512], BF, tag="ot")
                diag = list(range(4 * j, 4 * j + 4))
                hist = list(range(4 * j))
                if hist:
                    kb_order = []
                    step = max(1, len(hist) // 4)
                    hi = 0
                    for dkb in diag:
                        kb_order.append(dkb)
                        kb_order.extend(hist[hi:hi + step])
                        hi += step
                    kb_order.extend(hist[hi:])
                else:
                    kb_order = diag
                nkb = len(kb_order)
                LAG = 4
                hstate = {}
                pend = []
                norm_q = []
                rs_left = {}
                rs_started = {}
                hist_stage = {}
                diag_stage = {}

                def stage_rs(h, e, stage):
                    # sum groups of 4 e-tiles on VectorE (independent bf16
                    # adds); returns the completed group sum or None
                    st = stage[h]
                    st.append(e)
                    if len(st) == 2:
                        g = gp.tile([128, 512], BF, tag="g", name="g")
                        nc.vector.tensor_add(g[:], st[0][:], st[1][:])
                        st[1] = g
                    elif len(st) == 4:
                        g2 = gp.tile([128, 512], BF, tag="g", name="g2")
                        nc.vector.tensor_add(g2[:], st[2][:], st[3][:])
                        g = st[1]
                        nc.vector.tensor_add(g[:], g[:], g2[:])
                        stage[h] = []
                        return g
                    return None

                def flush_one():
                    # one rs matmul per 4-block group quarters the PE
                    # row-sum cost (diagonal tiles had their dead columns
                    # zeroed on GpSimd so they group full-width)
                    h, i, kb, e, off = pend.pop(0)
                    rs_ps, o_ps = hstate[h]
                    d = kb - 4 * j
                    g = stage_rs(h, e, diag_stage if d >= 0 else hist_stage)
                    if g is not None:
                        rs_left[h] -= 1
                        nc.tensor.matmul(rs_ps[:], ones[:], g[:],
                                         start=not rs_started[h],
                                         stop=(rs_left[h] == 0),
                                         skip_group_check=True)
                        rs_started[h] = True
                    nc.tensor.matmul(
                        o_ps[:, off:], V[:, kb, h * 128:(h + 1) * 128],
                        e[:, off:],
                        start=(i == 0), stop=(i == nkb - 1),
                        skip_group_check=True)
                    if i == nkb - 1:
                        norm_q.append(h)

                def emit_norm():
                    h = norm_q.pop(0)
                    rs_ps, o_ps = hstate[h]
                    rc = rcp.tile([128, 512], F, tag="rc")
                    nc.vector.reciprocal_approx_fast(rc[:], rs_ps[:])
                    # halves so the in-order VectorE queue can slip
                    # latency-critical mask ops between them
                    for q0 in range(0, 512, 256):
                        nc.vector.tensor_mul(ot[:, h, q0:q0 + 256],
                                             o_ps[:, q0:q0 + 256],
                                             rc[:, q0:q0 + 256])

                # with the fast reciprocal the norm chain is short; emitting
                # promptly releases the head's PSUM banks before the next
                # chunk's V projection needs them
                norm_keep = 0
                for h in range(HEADS_PER_CORE):
                    while len(norm_q) > norm_keep:
                        emit_norm()
                    hstate[h] = (
                        psB.tile([128, 512], F, tag="hold", name=f"rs{h}"),
                        psB.tile([128, 512], F, tag="hold", name=f"o{h}"))
                    rs_left[h] = 1 + j
                    rs_started[h] = False
                    hist_stage[h] = []
                    diag_stage[h] = []
                    for i, kb in enumerate(kb_order):
                        d = kb - 4 * j
                        off = 128 * d if d > 0 else 0
                        s_ps = psA.tile([128, 512], F, tag="flow")
                        nc.tensor.matmul(
                            s_ps[:, off:], KT[:, h, kb * 128:(kb + 1) * 128],
                            qt[:, h, off:], start=True, stop=True)
                        while len(pend) >= LAG:
                            flush_one()
                        e = ep.tile([128, 512], BF, tag="e")
                        if off > 0:
                            # zero the dead columns (GpSimd, idle) so the
                            # diagonal tiles can join a full-width rs group
                            nc.gpsimd.memset(e[:, :off], 0.0)
                        nc.scalar.activation(e[:, off:], s_ps[:, off:],
                                             AF.Exp, scale=SCALE)
                        if d >= 0:
                            # triangular 128-wide window at the diagonal
                            nc.vector.tensor_mul(
                                e[:, off:off + 128],
                                e[:, off:off + 128], mask_sb[:])
                        pend.append((h, i, kb, e, off))
                        if i == nkb - 2 and norm_q:
                            emit_norm()
                while pend:
                    flush_one()
                while norm_q:
                    emit_norm()
                prev_ot = ot

            # output projection for the last chunk; copies alternate between
            # VectorE and the now-idle ScalarE to halve the drain tail
            for t in range(16):
                emit_wo_step(N_CHUNKS - 1, prev_ot, t // 4, t % 4,
                             alt=(t % 2 == 1), last=True)

    nc.compile()
    return nc


def _get_program():
    if "nc" not in _CACHE:
        _CACHE["nc"] = _build_program()
    return _CACHE["nc"]


def _host_prep(x, token_positions, Wq, Wk, Wv, Wo):
    import ml_dtypes

    BF = ml_dtypes.bfloat16
    x = np.asarray(x, dtype=np.float32)
    Wq = np.asarray(Wq, dtype=np.float32)
    Wk = np.asarray(Wk, dtype=np.float32)
    Wv = np.asarray(Wv, dtype=np.float32)
    Wo = np.asarray(Wo, dtype=np.float32)
    pos = np.asarray(token_positions).astype(np.float64)

    # RoPE tables in permuted (half-split) layout, transposed to [d_k, s].
    inv = 10000.0 ** (-2.0 * np.arange(64, dtype=np.float64) / 128.0)
    ang = inv[:, None] * pos[None, :]  # [64, S]
    cos_h = np.cos(ang)
    sin_h = np.sin(ang)
    cosT = np.concatenate([cos_h, cos_h], axis=0).astype(np.float32)
    sinT = np.concatenate([-sin_h, sin_h], axis=0).astype(np.float32)

    # half-split permutation of each head's 128 feature columns
    perm = np.concatenate([np.arange(0, 128, 2), np.arange(1, 128, 2)])

    # triangular mask window: mask[p, w] = 1 iff p <= w (keys p, queries w
    # within the 128-wide diagonal window)
    mask = (np.arange(128)[:, None] <= np.arange(128)[None, :])
    mask = np.ascontiguousarray(mask).astype(BF)

    def permute_cols(W):  # [2048, 512] -> per-head column permutation
        return np.ascontiguousarray(
            W.reshape(D_MODEL, HEADS_PER_CORE, 128)[:, :, perm].reshape(
                D_MODEL, GCOLS))

    in_maps = []
    for core in range(N_CORES):
        b, g = divmod(core, 4)
        cols = slice(g * GCOLS, (g + 1) * GCOLS)
        in_maps.append({
            "xT": np.ascontiguousarray(x[b].T).astype(BF),
            "Wq": permute_cols(Wq[:, cols]).astype(BF),
            "Wk": permute_cols(Wk[:, cols]).astype(BF),
            "Wv": np.ascontiguousarray(Wv[:, cols]).astype(BF),
            "Wo": np.ascontiguousarray(Wo[cols, :]).astype(BF),
            "cosT": cosT,
            "sinT": sinT,
            "mask": mask,
        })
    return in_maps


def run_sharded(x, token_positions, Wq, Wk, Wv, Wo, trace=False, tmpdir=None):
    """Run the SPMD kernel; returns (full_output, BassKernelResults)."""
    from concourse import bass_utils

    nc = _get_program()
    in_maps = _host_prep(x, token_positions, Wq, Wk, Wv, Wo)
    kwargs = {}
    if trace:
        kwargs = {"trace": True, "tmpdir": tmpdir}
    res = bass_utils.run_bass_kernel_spmd(
        nc, in_maps, core_ids=list(range(N_CORES)), **kwargs)
    out = np.empty((BATCH, SEQ, D_MODEL), dtype=np.float32)
    for b in range(BATCH):
        acc = np.zeros((SEQ, D_MODEL), dtype=np.float64)
        for g in range(4):
            acc += res.results[b * 4 + g]["out"].astype(np.float32)
        out[b] = acc.astype(np.float32)
    return out, res


def kernel(x, token_positions, Wq, Wk, Wv, Wo):
    out, _ = run_sharded(x, token_positions, Wq, Wk, Wv, Wo)
    return out


# revision 31
# speedup vs baseline: 1.0015x; 1.0015x over previous
"""Causal multi-head attention (RoPE) TRN2 Bass kernel.

Problem: x[2,2048,2048] fp32, Wq/Wk/Wv/Wo [2048,2048], 16 heads, d_k=128,
causal softmax attention with interleaved RoPE, out = attn_out @ Wo.

Sharding (8 cores): core = b*4 + g handles batch b and head group g
(4 heads = 512 feature columns). Wq/Wk/Wv split column-wise, Wo row-wise;
the "all-reduce" after the output projection is done on the host by summing
the 4 partial outputs per batch (gather/unshard step).

All matmul operands are bf16 (inputs cast on the host, intermediates cast
at the PSUM drain): with bf16 weights the PE's LDWEIGHTS (~112ns) hides
under the 512-row matmul stream (~213ns), where the fp32r version's ~224ns
weight load serialized ~60ns per matmul. PSUM accumulation stays fp32.
Weights (Wq/Wk/Wv/Wo) and the RoPE tables are SBUF-resident, loaded once;
xT streams per 512-row chunk through a double-buffered tile (prefetched a
full chunk ahead).

Device kernel (per core), per 512-row chunk j:
  section 1: QT/KT = (x @ Wq/Wk)^T via lhsT=W tiles, rhs=xT, RoPE fused on
    VectorE; V = x @ Wv interleaved two k-steps per projection group; the
    PREVIOUS chunk's output projection (O @ Wo) interleaved as well.
  section 2: causal attention for q-tile j, scores computed transposed
    (S^T[k,q]) so softmax weights feed attn@V without transposes; exp on
    ScalarE (no max subtraction: scores are O(5)); rs/o accumulation
    matmuls trail the S matmuls by LAG slots in one flat software-pipelined
    stream across all four heads. Diagonal blocks are TRIMMED: for diagonal
    block d the first 128*d query columns are fully masked, so S/rs/o
    matmuls run on the live [128d:512] range only (bf16 keeps 1 cycle/row
    even at narrow widths) and no zero-fill is needed. Row sums: diagonal
    blocks use an all-ones lhsT matmul directly; history blocks are first
    summed in groups of 4 on VectorE (independent bf16 adds) with one
    ones-matmul per group, quartering the PE row-sum cost. 1/rs uses the
    single-op DVE reciprocal_approx_fast (~5x faster than the exact
    RECIPROCAL).

DMA: one Sync-queue stream for inputs in first-use order (weights land ~a
group ahead of their matmuls); mid-kernel output tiles ride the idle
GpSimd software-DGE queue; the final chunk's outputs use the Sync HWDGE
queue (idle by then, and its completion skips the slow SWDGE drain).

RoPE pair trick: scores are invariant under any permutation of d_k applied
to both Q and K, so W columns are permuted per head to [even..., odd...] on
the host; the rotate pairs then live 64 partitions apart (the sin-term
muls read ps at a 64-partition offset directly), and cosT/sinT are
permuted/sign-baked to match.
"""

import math
import sys

sys.path.insert(0, "/opt/trn_rl_repo")

import numpy as np

D_MODEL = 2048
SEQ = 2048
BATCH = 2
N_CORES = 8
HEADS_PER_CORE = 4
GCOLS = HEADS_PER_CORE * 128  # 512 feature columns per core
KB = D_MODEL // 128  # 16 contraction blocks
N_CHUNKS = SEQ // 512  # 4
SCALE = 1.0 / math.sqrt(128.0)

_CACHE = {}


def _build_program():
    import concourse.mybir as mybir
    import concourse.tile as tile
    from concourse import bacc

    F = mybir.dt.float32
    BF = mybir.dt.bfloat16
    AF = mybir.ActivationFunctionType

    nc = bacc.Bacc("TRN2", target_bir_lowering=False, debug=False,
                   num_devices=N_CORES)

    xT_d = nc.dram_tensor("xT", (D_MODEL, SEQ), BF, kind="ExternalInput").ap()
    Wq_d = nc.dram_tensor("Wq", (D_MODEL, GCOLS), BF, kind="ExternalInput").ap()
    Wk_d = nc.dram_tensor("Wk", (D_MODEL, GCOLS), BF, kind="ExternalInput").ap()
    Wv_d = nc.dram_tensor("Wv", (D_MODEL, GCOLS), BF, kind="ExternalInput").ap()
    Wo_d = nc.dram_tensor("Wo", (GCOLS, D_MODEL), BF, kind="ExternalInput").ap()
    cosT_d = nc.dram_tensor("cosT", (128, SEQ), F, kind="ExternalInput").ap()
    sinT_d = nc.dram_tensor("sinT", (128, SEQ), F, kind="ExternalInput").ap()
    mask_d = nc.dram_tensor("mask", (128, 128), BF, kind="ExternalInput").ap()
    # partial outputs are summed across 4 cores on the host (in f64), so
    # bf16 partials cost ~0.2% relative error but halve the output traffic
    # that dominates the end-of-kernel DMA drain.
    out_d = nc.dram_tensor("out", (SEQ, D_MODEL), BF, kind="ExternalOutput").ap()

    with tile.TileContext(nc) as tc:
        with tc.tile_pool(name="resid", bufs=1) as resid, \
             tc.tile_pool(name="xtp", bufs=2) as xtp, \
             tc.tile_pool(name="qtp", bufs=2) as qtp, \
             tc.tile_pool(name="otp", bufs=1) as otp, \
             tc.tile_pool(name="ep", bufs=12) as ep, \
             tc.tile_pool(name="gp", bufs=5) as gp, \
             tc.tile_pool(name="ropep", bufs=3) as ropep, \
             tc.tile_pool(name="rcp", bufs=2) as rcp, \
             tc.tile_pool(name="outp", bufs=3) as outp, \
             tc.tile_pool(name="psA", bufs=4, space="PSUM") as psA, \
             tc.tile_pool(name="psB", bufs=4, space="PSUM") as psB:

            ones = resid.tile([128, 128], BF, tag="ones")
            nc.vector.memset(ones[:], 1.0)
            mask_sb = resid.tile([128, 128], BF, tag="mask")
            KT = resid.tile([128, HEADS_PER_CORE, SEQ], BF, tag="KT")
            V = resid.tile([128, KB, GCOLS], BF, tag="V")
            wo = resid.tile([128, HEADS_PER_CORE, D_MODEL], BF, tag="wo")
            wq = resid.tile([128, KB, GCOLS], BF, tag="wq")
            wk = resid.tile([128, KB, GCOLS], BF, tag="wk")
            wv = resid.tile([128, KB, GCOLS], BF, tag="wv")
            cosT = resid.tile([128, SEQ], F, tag="cosT")
            sinT = resid.tile([128, SEQ], F, tag="sinT")

            xT_r = xT_d.rearrange("(ko p) s -> p ko s", p=128)
            Wq_r = Wq_d.rearrange("(ko p) m -> p ko m", p=128)
            Wk_r = Wk_d.rearrange("(ko p) m -> p ko m", p=128)
            Wv_r = Wv_d.rearrange("(ko p) m -> p ko m", p=128)

            def emit_wo_step(jprev, prev_ot, m, n, alt=False, last=False):
                ps = psA.tile([128, 512], F, tag="flow", name="wops")
                for c in range(HEADS_PER_CORE):
                    nc.tensor.matmul(
                        ps[:], prev_ot[:, c, m * 128:(m + 1) * 128],
                        wo[:, c, n * 512:(n + 1) * 512],
                        start=(c == 0), stop=(c == 3))
                ob = outp.tile([128, 512], BF, tag="ob")
                if alt:
                    nc.scalar.copy(ob[:], ps[:])
                else:
                    nc.vector.tensor_copy(ob[:], ps[:])
                # mid-kernel output DMAs ride the (otherwise idle) GpSimd
                # software-DGE queue so they never serialize behind the input
                # stream; the final chunk's go on the Sync HWDGE queue (idle
                # by then) whose completion doesn't need the slow SWDGE drain
                dst = out_d[(4 * jprev + m) * 128:(4 * jprev + m + 1) * 128,
                            n * 512:(n + 1) * 512]
                if last:
                    nc.sync.dma_start(dst, ob[:])
                else:
                    nc.gpsimd.dma_start(dst, ob[:])

            prev_ot = None

            for j in range(N_CHUNKS):
                ssl = slice(j * 512, (j + 1) * 512)

                if j == 0:
                    # startup: one queue, DMAs issued in first-use order so
                    # each QK group's weights land ~a group ahead of its
                    # matmuls (multiple queues lose this priority ordering)
                    xt = xtp.tile([128, KB, 512], BF, tag="xt")
                    nc.sync.dma_start(wq[:, 0:1, 0:128], Wq_r[:, 0:1, 0:128])
                    nc.sync.dma_start(xt[:, 0:1], xT_r[:, 0:1, ssl])
                    nc.sync.dma_start(wq[:, 1:KB, 0:128], Wq_r[:, 1:KB, 0:128])
                    nc.sync.dma_start(xt[:, 1:4], xT_r[:, 1:4, ssl])
                    nc.sync.dma_start(xt[:, 4:8], xT_r[:, 4:8, ssl])
                    nc.sync.dma_start(wv[:, 0:2], Wv_r[:, 0:2])
                    nc.sync.dma_start(cosT[:, 0:512], cosT_d[:, 0:512])
                    nc.sync.dma_start(sinT[:, 0:512], sinT_d[:, 0:512])
                    nc.sync.dma_start(xt[:, 8:KB], xT_r[:, 8:KB, ssl])
                    for g in range(1, 8):
                        wdst, wsrc = (wq, Wq_r) if g < 4 else (wk, Wk_r)
                        c0 = (g % 4) * 128
                        nc.sync.dma_start(wdst[:, :, c0:c0 + 128],
                                          wsrc[:, :, c0:c0 + 128])
                        nc.sync.dma_start(wv[:, 2 * g:2 * g + 2],
                                          Wv_r[:, 2 * g:2 * g + 2])
                    nc.sync.dma_start(mask_sb[:], mask_d)
                    nc.sync.dma_start(cosT[:, 512:SEQ], cosT_d[:, 512:SEQ])
                    nc.sync.dma_start(sinT[:, 512:SEQ], sinT_d[:, 512:SEQ])
                else:
                    xt = xt_next

                if j < N_CHUNKS - 1:
                    # prefetch next chunk's xT a full chunk ahead
                    xt_next = xtp.tile([128, KB, 512], BF, tag="xt")
                    nc.sync.dma_start(
                        xt_next[:], xT_r[:, :, (j + 1) * 512:(j + 2) * 512])

                cos_t = cosT[:, ssl]
                sin_t = sinT[:, ssl]
                qt = qtp.tile([128, HEADS_PER_CORE, 512], BF, tag="qt")

                # --- Q/K projections + RoPE (outputs transposed: [d_k, s]),
                # with the V projection's k-steps interleaved between groups
                # and the previous chunk's output projection as well. ---
                vps = [psB.tile([128, 512], F, tag="hold", name=f"vps{m}")
                       for m in range(4)]
                groups = [(qt, True, wq, m) for m in range(HEADS_PER_CORE)]
                groups += [(KT, False, wk, m) for m in range(HEADS_PER_CORE)]
                # group 0 carries no V/Wo work: the V matmuls (psB) and the
                # first Wo steps (prev_ot) would otherwise stall the in-order
                # PE queue on the previous chunk's last norms draining on
                # VectorE at the chunk boundary
                SCHED = [(), (0, 1), (2, 3), (4, 5), (6, 7), (8, 9),
                         (10, 11, 12), (13, 14, 15)]
                for g, (dst, is_q, w, m) in enumerate(groups):
                    msl = slice(m * 128, (m + 1) * 128)
                    ps = psA.tile([128, 512], F, tag="flow")
                    for k in range(KB):
                        nc.tensor.matmul(ps[:], w[:, k, msl], xt[:, k],
                                         start=(k == 0), stop=(k == KB - 1))
                    # V k-steps interleaved between groups
                    for k in SCHED[g]:
                        for m2 in range(4):
                            nc.tensor.matmul(
                                vps[m2][:],
                                xt[:, k, m2 * 128:(m2 + 1) * 128], wv[:, k],
                                start=(k == 0), stop=(k == KB - 1))
                    # Wo output-projection steps for the previous chunk
                    if prev_ot is not None:
                        for t in SCHED[g]:
                            emit_wo_step(j - 1, prev_ot, t // 4, t % 4)
                    # rotate-halves trick: the sin term reads ps with a
                    # 64-partition offset directly (no staging copies)
                    rot = ropep.tile([128, 512], F, tag="rot")
                    out_ap = dst[:, m, :] if is_q else dst[:, m, ssl]
                    nc.vector.tensor_mul(out_ap, ps[:], cos_t)
                    nc.vector.tensor_mul(rot[:64, :], ps[64:128, :],
                                         sin_t[:64, :])
                    nc.vector.tensor_mul(rot[64:128, :], ps[:64, :],
                                         sin_t[64:128, :])
                    nc.vector.tensor_add(out_ap, out_ap, rot[:])
                for m in range(4):
                    nc.any.tensor_copy(V[:, 4 * j + m, :], vps[m][:])

                if j == 0:
                    # Wo is first needed ~70us in; keep it off the startup
                    # critical path but loaded well before the Wo section.
                    nc.sync.dma_start(
                        wo[:], Wo_d.rearrange("(c p) n -> p c n", p=128))

                # --- causal attention for q-tile j ---
                # kb order per head: diagonal blocks first (their exp+mask
                # chain is the longest), then the fully-unmasked history
                # blocks. One flat software-pipelined stream across all four
                # heads: the rs/o accumulation matmuls for E(i) trail the S
                # matmuls by LAG slots. Diagonal block d only computes the
                # live query range [128d:512].
                ot = otp.tile([128, HEADS_PER_CORE, 512], BF, tag="ot")
                diag = list(range(4 * j, 4 * j + 4))
                hist = list(range(4 * j))
                if hist:
                    kb_order = []
                    step = max(1, len(hist) // 4)
                    hi = 0
                    for dkb in diag:
                        kb_order.append(dkb)
                        kb_order.extend(hist[hi:hi + step])
                        hi += step
                    kb_order.extend(hist[hi:])
                else:
                    kb_order = diag
                nkb = len(kb_order)
                LAG = 4
                hstate = {}
                pend = []
                norm_q = []
                rs_left = {}
                hist_stage = {}

                def flush_one():
                    h, i, kb, e, off = pend.pop(0)
                    rs_ps, o_ps = hstate[h]
                    d = kb - 4 * j
                    if d >= 0:
                        # diagonal block: direct (trimmed) rs matmul
                        rs_left[h] -= 1
                        nc.tensor.matmul(rs_ps[:, off:], ones[:], e[:, off:],
                                         start=(i == 0),
                                         stop=(rs_left[h] == 0),
                                         skip_group_check=True)
                    else:
                        # history blocks: sum groups of 4 e-tiles on VectorE
                        # (independent bf16 adds), one rs matmul per group —
                        # quarters the PE row-sum cost for the history part
                        st = hist_stage[h]
                        st.append(e)
                        if len(st) == 2:
                            g = gp.tile([128, 512], BF, tag="g", name="g")
                            nc.vector.tensor_add(g[:], st[0][:], st[1][:])
                            st[1] = g
                        elif len(st) == 4:
                            g2 = gp.tile([128, 512], BF, tag="g", name="g2")
                            nc.vector.tensor_add(g2[:], st[2][:], st[3][:])
                            g = st[1]
                            nc.vector.tensor_add(g[:], g[:], g2[:])
                            rs_left[h] -= 1
                            nc.tensor.matmul(rs_ps[:], ones[:], g[:],
                                             start=False,
                                             stop=(rs_left[h] == 0),
                                             skip_group_check=True)
                            hist_stage[h] = []
                    nc.tensor.matmul(
                        o_ps[:, off:], V[:, kb, h * 128:(h + 1) * 128],
                        e[:, off:],
                        start=(i == 0), stop=(i == nkb - 1),
                        skip_group_check=True)
                    if i == nkb - 1:
                        norm_q.append(h)

                def emit_norm():
                    h = norm_q.pop(0)
                    rs_ps, o_ps = hstate[h]
                    rc = rcp.tile([128, 512], F, tag="rc")
                    nc.vector.reciprocal_approx_fast(rc[:], rs_ps[:])
                    # halves so the in-order VectorE queue can slip
                    # latency-critical mask ops between them
                    for q0 in range(0, 512, 256):
                        nc.vector.tensor_mul(ot[:, h, q0:q0 + 256],
                                             o_ps[:, q0:q0 + 256],
                                             rc[:, q0:q0 + 256])

                # with the fast reciprocal the norm chain is short; emitting
                # promptly releases the head's PSUM banks before the next
                # chunk's V projection needs them
                norm_keep = 0
                for h in range(HEADS_PER_CORE):
                    while len(norm_q) > norm_keep:
                        emit_norm()
                    hstate[h] = (
                        psB.tile([128, 512], F, tag="hold", name=f"rs{h}"),
                        psB.tile([128, 512], F, tag="hold", name=f"o{h}"))
                    rs_left[h] = 4 + j
                    hist_stage[h] = []
                    for i, kb in enumerate(kb_order):
                        d = kb - 4 * j
                        off = 128 * d if d > 0 else 0
                        s_ps = psA.tile([128, 512], F, tag="flow")
                        nc.tensor.matmul(
                            s_ps[:, off:], KT[:, h, kb * 128:(kb + 1) * 128],
                            qt[:, h, off:], start=True, stop=True)
                        while len(pend) >= LAG:
                            flush_one()
                        e = ep.tile([128, 512], BF, tag="e")
                        nc.scalar.activation(e[:, off:], s_ps[:, off:],
                                             AF.Exp, scale=SCALE)
                        if d >= 0:
                            # triangular 128-wide window at the diagonal
                            nc.vector.tensor_mul(
                                e[:, off:off + 128],
                                e[:, off:off + 128], mask_sb[:])
                        pend.append((h, i, kb, e, off))
                        if i == nkb - 2 and norm_q:
                            emit_norm()
                while pend:
                    flush_one()
                while norm_q:
                    emit_norm()
                prev_ot = ot

            # output projection for the last chunk; copies alternate between
            # VectorE and the now-idle ScalarE to halve the drain tail
            for t in range(16):
                emit_wo_step(N_CHUNKS - 1, prev_ot, t // 4, t % 4,
                             alt=(t % 2 == 1), last=True)

    nc.compile()
    return nc


def _get_program():
    if "nc" not in _CACHE:
        _CACHE["nc"] = _build_program()
    return _CACHE["nc"]


def _host_prep(x, token_positions, Wq, Wk, Wv, Wo):
    import ml_dtypes

    BF = ml_dtypes.bfloat16
    x = np.asarray(x, dtype=np.float32)
    Wq = np.asarray(Wq, dtype=np.float32)
    Wk = np.asarray(Wk, dtype=np.float32)
    Wv = np.asarray(Wv, dtype=np.float32)
    Wo = np.asarray(Wo, dtype=np.float32)
    pos = np.asarray(token_positions).astype(np.float64)

    # RoPE tables in permuted (half-split) layout, transposed to [d_k, s].
    inv = 10000.0 ** (-2.0 * np.arange(64, dtype=np.float64) / 128.0)
    ang = inv[:, None] * pos[None, :]  # [64, S]
    cos_h = np.cos(ang)
    sin_h = np.sin(ang)
    cosT = np.concatenate([cos_h, cos_h], axis=0).astype(np.float32)
    sinT = np.concatenate([-sin_h, sin_h], axis=0).astype(np.float32)

    # half-split permutation of each head's 128 feature columns
    perm = np.concatenate([np.arange(0, 128, 2), np.arange(1, 128, 2)])

    # triangular mask window: mask[p, w] = 1 iff p <= w (keys p, queries w
    # within the 128-wide diagonal window)
    mask = (np.arange(128)[:, None] <= np.arange(128)[None, :])
    mask = np.ascontiguousarray(mask).astype(BF)

    def permute_cols(W):  # [2048, 512] -> per-head column permutation
        return np.ascontiguousarray(
            W.reshape(D_MODEL, HEADS_PER_CORE, 128)[:, :, perm].reshape(
                D_MODEL, GCOLS))

    in_maps = []
    for core in range(N_CORES):
        b, g = divmod(core, 4)
        cols = slice(g * GCOLS, (g + 1) * GCOLS)
        in_maps.append({
            "xT": np.ascontiguousarray(x[b].T).astype(BF),
            "Wq": permute_cols(Wq[:, cols]).astype(BF),
            "Wk": permute_cols(Wk[:, cols]).astype(BF),
            "Wv": np.ascontiguousarray(Wv[:, cols]).astype(BF),
            "Wo": np.ascontiguousarray(Wo[cols, :]).astype(BF),
            "cosT": cosT,
            "sinT": sinT,
            "mask": mask,
        })
    return in_maps


def run_sharded(x, token_positions, Wq, Wk, Wv, Wo, trace=False, tmpdir=None):
    """Run the SPMD kernel; returns (full_output, BassKernelResults)."""
    from concourse import bass_utils

    nc = _get_program()
    in_maps = _host_prep(x, token_positions, Wq, Wk, Wv, Wo)
    kwargs = {}
    if trace:
        kwargs = {"trace": True, "tmpdir": tmpdir}
    res = bass_utils.run_bass_kernel_spmd(
        nc, in_maps, core_ids=list(range(N_CORES)), **kwargs)
    out = np.empty((BATCH, SEQ, D_MODEL), dtype=np.float32)
    for b in range(BATCH):
        acc = np.zeros((SEQ, D_MODEL), dtype=np.float64)
        for g in range(4):
            acc += res.results[b * 4 + g]["out"].astype(np.float32)
        out[b] = acc.astype(np.float32)
    return out, res


def kernel(x, token_positions, Wq, Wk, Wv, Wo):
    out, _ = run_sharded(x, token_positions, Wq, Wk, Wv, Wo)
    return out


# revision 32
# speedup vs baseline: 1.0084x; 1.0069x over previous
"""Causal multi-head attention (RoPE) TRN2 Bass kernel.

Problem: x[2,2048,2048] fp32, Wq/Wk/Wv/Wo [2048,2048], 16 heads, d_k=128,
causal softmax attention with interleaved RoPE, out = attn_out @ Wo.

Sharding (8 cores): core = b*4 + g handles batch b and head group g
(4 heads = 512 feature columns). Wq/Wk/Wv split column-wise, Wo row-wise;
the "all-reduce" after the output projection is done on the host by summing
the 4 partial outputs per batch (gather/unshard step).

All matmul operands are bf16 (inputs cast on the host, intermediates cast
at the PSUM drain): with bf16 weights the PE's LDWEIGHTS (~112ns) hides
under the 512-row matmul stream (~213ns), where the fp32r version's ~224ns
weight load serialized ~60ns per matmul. PSUM accumulation stays fp32.
Weights (Wq/Wk/Wv/Wo) and the RoPE tables are SBUF-resident, loaded once;
xT streams per 512-row chunk through a double-buffered tile (prefetched a
full chunk ahead).

Device kernel (per core), per 512-row chunk j:
  section 1: QT/KT = (x @ Wq/Wk)^T via lhsT=W tiles, rhs=xT, RoPE fused on
    VectorE; V = x @ Wv interleaved two k-steps per projection group; the
    PREVIOUS chunk's output projection (O @ Wo) interleaved as well.
  section 2: causal attention for q-tile j, scores computed transposed
    (S^T[k,q]) so softmax weights feed attn@V without transposes; exp on
    ScalarE (no max subtraction: scores are O(5)); rs/o accumulation
    matmuls trail the S matmuls by LAG slots in one flat software-pipelined
    stream across all four heads. Diagonal blocks are TRIMMED: for diagonal
    block d the first 128*d query columns are fully masked, so S/rs/o
    matmuls run on the live [128d:512] range only (bf16 keeps 1 cycle/row
    even at narrow widths) and no zero-fill is needed. Row sums: diagonal
    blocks use an all-ones lhsT matmul directly; history blocks are first
    summed in groups of 4 on VectorE (independent bf16 adds) with one
    ones-matmul per group, quartering the PE row-sum cost. 1/rs uses the
    single-op DVE reciprocal_approx_fast (~5x faster than the exact
    RECIPROCAL).

DMA: one Sync-queue stream for inputs in first-use order (weights land ~a
group ahead of their matmuls); mid-kernel output tiles ride the idle
GpSimd software-DGE queue; the final chunk's outputs use the Sync HWDGE
queue (idle by then, and its completion skips the slow SWDGE drain).

RoPE pair trick: scores are invariant under any permutation of d_k applied
to both Q and K, so W columns are permuted per head to [even..., odd...] on
the host; the rotate pairs then live 64 partitions apart (the sin-term
muls read ps at a 64-partition offset directly), and cosT/sinT are
permuted/sign-baked to match.
"""

import math
import sys

sys.path.insert(0, "/opt/trn_rl_repo")

import numpy as np

D_MODEL = 2048
SEQ = 2048
BATCH = 2
N_CORES = 8
HEADS_PER_CORE = 4
GCOLS = HEADS_PER_CORE * 128  # 512 feature columns per core
KB = D_MODEL // 128  # 16 contraction blocks
N_CHUNKS = SEQ // 512  # 4
SCALE = 1.0 / math.sqrt(128.0)

_CACHE = {}


def _build_program():
    import concourse.mybir as mybir
    import concourse.tile as tile
    from concourse import bacc

    F = mybir.dt.float32
    BF = mybir.dt.bfloat16
    AF = mybir.ActivationFunctionType

    nc = bacc.Bacc("TRN2", target_bir_lowering=False, debug=False,
                   num_devices=N_CORES)

    xT_d = nc.dram_tensor("xT", (D_MODEL, SEQ), BF, kind="ExternalInput").ap()
    Wq_d = nc.dram_tensor("Wq", (D_MODEL, GCOLS), BF, kind="ExternalInput").ap()
    Wk_d = nc.dram_tensor("Wk", (D_MODEL, GCOLS), BF, kind="ExternalInput").ap()
    Wv_d = nc.dram_tensor("Wv", (D_MODEL, GCOLS), BF, kind="ExternalInput").ap()
    Wo_d = nc.dram_tensor("Wo", (GCOLS, D_MODEL), BF, kind="ExternalInput").ap()
    cosT_d = nc.dram_tensor("cosT", (128, SEQ), F, kind="ExternalInput").ap()
    sinT_d = nc.dram_tensor("sinT", (128, SEQ), F, kind="ExternalInput").ap()
    mask_d = nc.dram_tensor("mask", (128, 128), BF, kind="ExternalInput").ap()
    # partial outputs are summed across 4 cores on the host (in f64), so
    # bf16 partials cost ~0.2% relative error but halve the output traffic
    # that dominates the end-of-kernel DMA drain.
    out_d = nc.dram_tensor("out", (SEQ, D_MODEL), BF, kind="ExternalOutput").ap()

    with tile.TileContext(nc) as tc:
        with tc.tile_pool(name="resid", bufs=1) as resid, \
             tc.tile_pool(name="xtp", bufs=2) as xtp, \
             tc.tile_pool(name="qtp", bufs=2) as qtp, \
             tc.tile_pool(name="otp", bufs=1) as otp, \
             tc.tile_pool(name="ep", bufs=12) as ep, \
             tc.tile_pool(name="gp", bufs=5) as gp, \
             tc.tile_pool(name="ropep", bufs=3) as ropep, \
             tc.tile_pool(name="rcp", bufs=2) as rcp, \
             tc.tile_pool(name="outp", bufs=3) as outp, \
             tc.tile_pool(name="psA", bufs=4, space="PSUM") as psA, \
             tc.tile_pool(name="psB", bufs=4, space="PSUM") as psB:

            ones = resid.tile([128, 128], BF, tag="ones")
            nc.vector.memset(ones[:], 1.0)
            mask_sb = resid.tile([128, 128], BF, tag="mask")
            KT = resid.tile([128, HEADS_PER_CORE, SEQ], BF, tag="KT")
            V = resid.tile([128, KB, GCOLS], BF, tag="V")
            wo = resid.tile([128, HEADS_PER_CORE, D_MODEL], BF, tag="wo")
            wq = resid.tile([128, KB, GCOLS], BF, tag="wq")
            wk = resid.tile([128, KB, GCOLS], BF, tag="wk")
            wv = resid.tile([128, KB, GCOLS], BF, tag="wv")
            cosT = resid.tile([128, SEQ], F, tag="cosT")
            sinT = resid.tile([128, SEQ], F, tag="sinT")

            xT_r = xT_d.rearrange("(ko p) s -> p ko s", p=128)
            Wq_r = Wq_d.rearrange("(ko p) m -> p ko m", p=128)
            Wk_r = Wk_d.rearrange("(ko p) m -> p ko m", p=128)
            Wv_r = Wv_d.rearrange("(ko p) m -> p ko m", p=128)

            def emit_wo_step(jprev, prev_ot, m, n, alt=False, last=False):
                ps = psA.tile([128, 512], F, tag="flow", name="wops")
                for c in range(HEADS_PER_CORE):
                    nc.tensor.matmul(
                        ps[:], prev_ot[:, c, m * 128:(m + 1) * 128],
                        wo[:, c, n * 512:(n + 1) * 512],
                        start=(c == 0), stop=(c == 3))
                ob = outp.tile([128, 512], BF, tag="ob")
                if alt:
                    nc.scalar.copy(ob[:], ps[:])
                else:
                    nc.vector.tensor_copy(ob[:], ps[:])
                # mid-kernel output DMAs ride the (otherwise idle) GpSimd
                # software-DGE queue so they never serialize behind the input
                # stream; the final chunk's go on the Sync HWDGE queue (idle
                # by then) whose completion doesn't need the slow SWDGE drain
                dst = out_d[(4 * jprev + m) * 128:(4 * jprev + m + 1) * 128,
                            n * 512:(n + 1) * 512]
                if last:
                    nc.sync.dma_start(dst, ob[:])
                else:
                    nc.gpsimd.dma_start(dst, ob[:])

            prev_ot = None

            for j in range(N_CHUNKS):
                ssl = slice(j * 512, (j + 1) * 512)

                if j == 0:
                    # startup: one queue, DMAs issued in first-use order so
                    # each QK group's weights land ~a group ahead of its
                    # matmuls (multiple queues lose this priority ordering)
                    xt = xtp.tile([128, KB, 512], BF, tag="xt")
                    nc.sync.dma_start(wq[:, 0:1, 0:128], Wq_r[:, 0:1, 0:128])
                    nc.sync.dma_start(xt[:, 0:1], xT_r[:, 0:1, ssl])
                    nc.sync.dma_start(wq[:, 1:KB, 0:128], Wq_r[:, 1:KB, 0:128])
                    nc.sync.dma_start(xt[:, 1:4], xT_r[:, 1:4, ssl])
                    nc.sync.dma_start(xt[:, 4:8], xT_r[:, 4:8, ssl])
                    nc.sync.dma_start(wv[:, 0:2], Wv_r[:, 0:2])
                    nc.sync.dma_start(cosT[:, 0:512], cosT_d[:, 0:512])
                    nc.sync.dma_start(sinT[:, 0:512], sinT_d[:, 0:512])
                    nc.sync.dma_start(xt[:, 8:KB], xT_r[:, 8:KB, ssl])
                    for g in range(1, 8):
                        wdst, wsrc = (wq, Wq_r) if g < 4 else (wk, Wk_r)
                        c0 = (g % 4) * 128
                        nc.sync.dma_start(wdst[:, :, c0:c0 + 128],
                                          wsrc[:, :, c0:c0 + 128])
                        nc.sync.dma_start(wv[:, 2 * g:2 * g + 2],
                                          Wv_r[:, 2 * g:2 * g + 2])
                    nc.sync.dma_start(mask_sb[:], mask_d)
                    nc.sync.dma_start(cosT[:, 512:SEQ], cosT_d[:, 512:SEQ])
                    nc.sync.dma_start(sinT[:, 512:SEQ], sinT_d[:, 512:SEQ])
                else:
                    xt = xt_next

                if j < N_CHUNKS - 1:
                    # prefetch next chunk's xT a full chunk ahead
                    xt_next = xtp.tile([128, KB, 512], BF, tag="xt")
                    nc.sync.dma_start(
                        xt_next[:], xT_r[:, :, (j + 1) * 512:(j + 2) * 512])

                cos_t = cosT[:, ssl]
                sin_t = sinT[:, ssl]
                qt = qtp.tile([128, HEADS_PER_CORE, 512], BF, tag="qt")

                # --- Q/K projections + RoPE (outputs transposed: [d_k, s]),
                # with the V projection's k-steps interleaved between groups
                # and the previous chunk's output projection as well. ---
                vps = [psB.tile([128, 512], F, tag="hold", name=f"vps{m}")
                       for m in range(4)]
                groups = [(qt, True, wq, m) for m in range(HEADS_PER_CORE)]
                groups += [(KT, False, wk, m) for m in range(HEADS_PER_CORE)]
                # group 0 carries no V/Wo work: the V matmuls (psB) and the
                # first Wo steps (prev_ot) would otherwise stall the in-order
                # PE queue on the previous chunk's last norms draining on
                # VectorE at the chunk boundary
                SCHED = [(), (0, 1), (2, 3), (4, 5), (6, 7), (8, 9),
                         (10, 11, 12), (13, 14, 15)]
                for g, (dst, is_q, w, m) in enumerate(groups):
                    msl = slice(m * 128, (m + 1) * 128)
                    ps = psA.tile([128, 512], F, tag="flow")
                    for k in range(KB):
                        nc.tensor.matmul(ps[:], w[:, k, msl], xt[:, k],
                                         start=(k == 0), stop=(k == KB - 1))
                    # V k-steps interleaved between groups
                    for k in SCHED[g]:
                        for m2 in range(4):
                            nc.tensor.matmul(
                                vps[m2][:],
                                xt[:, k, m2 * 128:(m2 + 1) * 128], wv[:, k],
                                start=(k == 0), stop=(k == KB - 1))
                    # Wo output-projection steps for the previous chunk
                    if prev_ot is not None:
                        for t in SCHED[g]:
                            emit_wo_step(j - 1, prev_ot, t // 4, t % 4)
                    # rotate-halves trick: the sin term reads ps with a
                    # 64-partition offset directly (no staging copies)
                    rot = ropep.tile([128, 512], F, tag="rot")
                    out_ap = dst[:, m, :] if is_q else dst[:, m, ssl]
                    nc.vector.tensor_mul(out_ap, ps[:], cos_t)
                    nc.vector.tensor_mul(rot[:64, :], ps[64:128, :],
                                         sin_t[:64, :])
                    nc.vector.tensor_mul(rot[64:128, :], ps[:64, :],
                                         sin_t[64:128, :])
                    # the final RoPE add is slack (consumed ~a section later)
                    # and both operands live in SBUF: the idle GpSimd engine
                    # takes it, thinning VectorE at the chunk boundary
                    nc.gpsimd.tensor_add(out_ap, out_ap, rot[:])
                for m in range(4):
                    nc.any.tensor_copy(V[:, 4 * j + m, :], vps[m][:])

                if j == 0:
                    # Wo is first needed ~70us in; keep it off the startup
                    # critical path but loaded well before the Wo section.
                    nc.sync.dma_start(
                        wo[:], Wo_d.rearrange("(c p) n -> p c n", p=128))

                # --- causal attention for q-tile j ---
                # kb order per head: diagonal blocks first (their exp+mask
                # chain is the longest), then the fully-unmasked history
                # blocks. One flat software-pipelined stream across all four
                # heads: the rs/o accumulation matmuls for E(i) trail the S
                # matmuls by LAG slots. Diagonal block d only computes the
                # live query range [128d:512].
                ot = otp.tile([128, HEADS_PER_CORE, 512], BF, tag="ot")
                diag = list(range(4 * j, 4 * j + 4))
                hist = list(range(4 * j))
                if hist:
                    kb_order = []
                    step = max(1, len(hist) // 4)
                    hi = 0
                    for dkb in diag:
                        kb_order.append(dkb)
                        kb_order.extend(hist[hi:hi + step])
                        hi += step
                    kb_order.extend(hist[hi:])
                else:
                    kb_order = diag
                nkb = len(kb_order)
                LAG = 4
                hstate = {}
                pend = []
                norm_q = []
                rs_left = {}
                hist_stage = {}

                def flush_one():
                    h, i, kb, e, off = pend.pop(0)
                    rs_ps, o_ps = hstate[h]
                    d = kb - 4 * j
                    if d >= 0:
                        # diagonal block: direct (trimmed) rs matmul
                        rs_left[h] -= 1
                        nc.tensor.matmul(rs_ps[:, off:], ones[:], e[:, off:],
                                         start=(i == 0),
                                         stop=(rs_left[h] == 0),
                                         skip_group_check=True)
                    else:
                        # history blocks: sum groups of 4 e-tiles on VectorE
                        # (independent bf16 adds), one rs matmul per group —
                        # quarters the PE row-sum cost for the history part
                        st = hist_stage[h]
                        st.append(e)
                        if len(st) == 2:
                            g = gp.tile([128, 512], BF, tag="g", name="g")
                            nc.vector.tensor_add(g[:], st[0][:], st[1][:])
                            st[1] = g
                        elif len(st) == 4:
                            g2 = gp.tile([128, 512], BF, tag="g", name="g2")
                            nc.vector.tensor_add(g2[:], st[2][:], st[3][:])
                            g = st[1]
                            nc.vector.tensor_add(g[:], g[:], g2[:])
                            rs_left[h] -= 1
                            nc.tensor.matmul(rs_ps[:], ones[:], g[:],
                                             start=False,
                                             stop=(rs_left[h] == 0),
                                             skip_group_check=True)
                            hist_stage[h] = []
                    nc.tensor.matmul(
                        o_ps[:, off:], V[:, kb, h * 128:(h + 1) * 128],
                        e[:, off:],
                        start=(i == 0), stop=(i == nkb - 1),
                        skip_group_check=True)
                    if i == nkb - 1:
                        norm_q.append(h)

                def emit_norm():
                    h = norm_q.pop(0)
                    rs_ps, o_ps = hstate[h]
                    rc = rcp.tile([128, 512], F, tag="rc")
                    nc.vector.reciprocal_approx_fast(rc[:], rs_ps[:])
                    # halves so the in-order VectorE queue can slip
                    # latency-critical mask ops between them
                    for q0 in range(0, 512, 256):
                        nc.vector.tensor_mul(ot[:, h, q0:q0 + 256],
                                             o_ps[:, q0:q0 + 256],
                                             rc[:, q0:q0 + 256])

                # with the fast reciprocal the norm chain is short; emitting
                # promptly releases the head's PSUM banks before the next
                # chunk's V projection needs them
                norm_keep = 0
                for h in range(HEADS_PER_CORE):
                    while len(norm_q) > norm_keep:
                        emit_norm()
                    hstate[h] = (
                        psB.tile([128, 512], F, tag="hold", name=f"rs{h}"),
                        psB.tile([128, 512], F, tag="hold", name=f"o{h}"))
                    rs_left[h] = 4 + j
                    hist_stage[h] = []
                    for i, kb in enumerate(kb_order):
                        d = kb - 4 * j
                        off = 128 * d if d > 0 else 0
                        s_ps = psA.tile([128, 512], F, tag="flow")
                        nc.tensor.matmul(
                            s_ps[:, off:], KT[:, h, kb * 128:(kb + 1) * 128],
                            qt[:, h, off:], start=True, stop=True)
                        while len(pend) >= LAG:
                            flush_one()
                        e = ep.tile([128, 512], BF, tag="e")
                        nc.scalar.activation(e[:, off:], s_ps[:, off:],
                                             AF.Exp, scale=SCALE)
                        if d >= 0:
                            # triangular 128-wide window at the diagonal
                            nc.vector.tensor_mul(
                                e[:, off:off + 128],
                                e[:, off:off + 128], mask_sb[:])
                        pend.append((h, i, kb, e, off))
                        if i == nkb - 2 and norm_q:
                            emit_norm()
                while pend:
                    flush_one()
                while norm_q:
                    emit_norm()
                prev_ot = ot

            # output projection for the last chunk; copies alternate between
            # VectorE and the now-idle ScalarE to halve the drain tail
            for t in range(16):
                emit_wo_step(N_CHUNKS - 1, prev_ot, t // 4, t % 4,
                             alt=(t % 2 == 1), last=True)

    nc.compile()
    return nc


def _get_program():
    if "nc" not in _CACHE:
        _CACHE["nc"] = _build_program()
    return _CACHE["nc"]


def _host_prep(x, token_positions, Wq, Wk, Wv, Wo):
    import ml_dtypes

    BF = ml_dtypes.bfloat16
    x = np.asarray(x, dtype=np.float32)
    Wq = np.asarray(Wq, dtype=np.float32)
    Wk = np.asarray(Wk, dtype=np.float32)
    Wv = np.asarray(Wv, dtype=np.float32)
    Wo = np.asarray(Wo, dtype=np.float32)
    pos = np.asarray(token_positions).astype(np.float64)

    # RoPE tables in permuted (half-split) layout, transposed to [d_k, s].
    inv = 10000.0 ** (-2.0 * np.arange(64, dtype=np.float64) / 128.0)
    ang = inv[:, None] * pos[None, :]  # [64, S]
    cos_h = np.cos(ang)
    sin_h = np.sin(ang)
    cosT = np.concatenate([cos_h, cos_h], axis=0).astype(np.float32)
    sinT = np.concatenate([-sin_h, sin_h], axis=0).astype(np.float32)

    # half-split permutation of each head's 128 feature columns
    perm = np.concatenate([np.arange(0, 128, 2), np.arange(1, 128, 2)])

    # triangular mask window: mask[p, w] = 1 iff p <= w (keys p, queries w
    # within the 128-wide diagonal window)
    mask = (np.arange(128)[:, None] <= np.arange(128)[None, :])
    mask = np.ascontiguousarray(mask).astype(BF)

    def permute_cols(W):  # [2048, 512] -> per-head column permutation
        return np.ascontiguousarray(
            W.reshape(D_MODEL, HEADS_PER_CORE, 128)[:, :, perm].reshape(
                D_MODEL, GCOLS))

    in_maps = []
    for core in range(N_CORES):
        b, g = divmod(core, 4)
        cols = slice(g * GCOLS, (g + 1) * GCOLS)
        in_maps.append({
            "xT": np.ascontiguousarray(x[b].T).astype(BF),
            "Wq": permute_cols(Wq[:, cols]).astype(BF),
            "Wk": permute_cols(Wk[:, cols]).astype(BF),
            "Wv": np.ascontiguousarray(Wv[:, cols]).astype(BF),
            "Wo": np.ascontiguousarray(Wo[cols, :]).astype(BF),
            "cosT": cosT,
            "sinT": sinT,
            "mask": mask,
        })
    return in_maps


def run_sharded(x, token_positions, Wq, Wk, Wv, Wo, trace=False, tmpdir=None):
    """Run the SPMD kernel; returns (full_output, BassKernelResults)."""
    from concourse import bass_utils

    nc = _get_program()
    in_maps = _host_prep(x, token_positions, Wq, Wk, Wv, Wo)
    kwargs = {}
    if trace:
        kwargs = {"trace": True, "tmpdir": tmpdir}
    res = bass_utils.run_bass_kernel_spmd(
        nc, in_maps, core_ids=list(range(N_CORES)), **kwargs)
    out = np.empty((BATCH, SEQ, D_MODEL), dtype=np.float32)
    for b in range(BATCH):
        acc = np.zeros((SEQ, D_MODEL), dtype=np.float64)
        for g in range(4):
            acc += res.results[b * 4 + g]["out"].astype(np.float32)
        out[b] = acc.astype(np.float32)
    return out, res


def kernel(x, token_positions, Wq, Wk, Wv, Wo):
    out, _ = run_sharded(x, token_positions, Wq, Wk, Wv, Wo)
    return out
